# revision 30
# baseline (speedup 1.0000x reference)
"""Multi-Head Latent Attention (MLA) prefill kernel for 8 Trainium2 NeuronCores.

Sharding: latent down-projections row-split 8 ways + AllGather; up-projections
and attention head-split (2 heads/core); AllToAll converts head-split attention
output to token-split for the output projection. Host only slices inputs and
concatenates the per-core output row slabs.

This revision batches HBM traffic into a handful of large DMAs (the sequencer
spends ~600ns configuring DGE per dma_start, which saturated SP in the original
version), moves weights/latents/attention tensors to bf16, and replaces the
partition-shuffle RoPE combine with signed-permutation matmuls on the PE.
"""
import sys
if '/opt/trn_rl_repo' not in sys.path:
    sys.path.insert(0, '/opt/trn_rl_repo')

import math
import numpy as np
import ml_dtypes

import concourse.bass as bass
import concourse.tile as tile
import concourse.mybir as mybir
from concourse import bacc

F32 = mybir.dt.float32
F32R = mybir.dt.float32r
BF16 = mybir.dt.bfloat16
FP8 = mybir.dt.float8e4
AF = mybir.ActivationFunctionType
ALU = mybir.AluOpType
NPBF16 = ml_dtypes.bfloat16

B, S, DIM, H = 2, 2048, 2048, 16
NOPE, ROPE, QKD, VD = 128, 64, 192, 128
QLR, KVLR = 512, 512
EPS = 1e-6
NC = 8
N = B * S              # 4096 flattened tokens
R = N // NC            # 512 tokens per core (phase 1 / output rows)
HPC = H // NC          # 2 heads per core
NBLK = N // R          # 8 token blocks (= AG shards)
SCALE = 1.0 / math.sqrt(QKD)
LATM = 9               # latent chunks in the AllGather payload (4 q + 4 kv + rope)
KD = DIM // 128        # 16 contraction chunks for the down-projections

SKIP, PLAIN = -2, -1   # mask block classes (>=0 -> index into mask blocks)


def _rope_tables():
    freqs = (1.0 / (10000.0 ** (np.arange(0, ROPE, 2, dtype=np.float32) / ROPE)))
    ang = np.arange(S, dtype=np.float32)[:, None] * freqs[None, :]      # [S, 32]
    return np.cos(ang).T.copy(), np.sin(ang).T.copy()                   # [32, S]


def _perm_mats(rows):
    """Signed-permutation matrices A, B with rot = A.T @ u + B.T @ w, where
    u = q*[c;s;...], w = q*[s;c;...] (32-row blocks) and rot is interleaved
    rope output [e;o;...].  rot[0:32] = u[0:32]-u[32:64];
    rot[32:64] = w[0:32]+w[32:64]; repeating every 64 rows."""
    A = np.zeros((rows, rows), np.float32)
    Bm = np.zeros((rows, rows), np.float32)
    for base in range(0, rows, 64):
        for i in range(32):
            A[base + i, base + i] = 1.0
            A[base + 32 + i, base + i] = -1.0
            Bm[base + i, base + 32 + i] = 1.0
            Bm[base + 32 + i, base + 32 + i] = 1.0
    return A, Bm


def _classify_mask(mask):
    """Per (q-chunk of 512, k-block of 128): SKIP / PLAIN / index of mask block.

    Returns (cls[4][16], blocks [nblk,128,512] transposed mask as bf16)."""
    cls = [[PLAIN] * (S // 128) for _ in range(S // 512)]
    blocks = []
    uniq = {}
    for qc in range(S // 512):
        sub_q = mask[qc * 512:(qc + 1) * 512]
        for kb in range(S // 128):
            blk = sub_q[:, kb * 128:(kb + 1) * 128]
            if not blk.any():
                cls[qc][kb] = SKIP
            elif blk.all():
                cls[qc][kb] = PLAIN
            else:
                bt = blk.T.astype(NPBF16)             # [128 k, 512 q]
                key = bt.tobytes()
                if key not in uniq:
                    uniq[key] = len(blocks)
                    blocks.append(bt)
                cls[qc][kb] = uniq[key]
    blocks = (np.stack(blocks) if blocks
              else np.zeros((0, 128, 512), NPBF16))
    return cls, blocks


def _build(cls, nmask, flags, repeat=1, sim_mode=False):
    """Emit the bass program. cls/nmask/flags are compile-time schedule data."""
    nc = bacc.Bacc(None, num_devices=NC)

    # ---- I/O ----
    x_c = nc.dram_tensor("x_c", [KD, 128, R], BF16, kind="ExternalInput")
    # down-proj weights, packed [k, part, 512 q | 512 kv | 64 rope] bf16
    w1 = nc.dram_tensor("w1", [KD, 128, QLR + KVLR + ROPE], BF16,
                        kind="ExternalInput")
    # biases for down-proj as [128, 9] (chunk-major), zero cols when absent
    b1 = nc.dram_tensor("b1", [128, LATM], F32, kind="ExternalInput")
    # rmsnorm weights as rows [1, 8*128]
    normw = nc.dram_tensor("normw", [1, 8 * 128], F32, kind="ExternalInput")
    # trig tables: phase-1 (this core's 512 positions) & phase-2 (full 2048)
    trig1 = nc.dram_tensor("trig1", [64, 2 * R], F32, kind="ExternalInput")  # [c;s],[s;c]
    trigq = nc.dram_tensor("trigq", [128, 2 * S], BF16, kind="ExternalInput")  # [cscs | scsc]
    abperm = nc.dram_tensor("abperm", [128, 256], BF16, kind="ExternalInput")
    # per-core up-proj weights, packed [k, part, 3*128 q | 2*128 k | 256 v] bf16
    w2 = nc.dram_tensor("w2", [4, 128, 5 * 128 + HPC * VD], BF16,
                        kind="ExternalInput")
    b2 = nc.dram_tensor("b2", [128, 5], F32, kind="ExternalInput")
    bvb = nc.dram_tensor("bvb", [1, HPC * VD], F32, kind="ExternalInput")
    # output projection weights [k(head-slice), part(vd), 2048] bf16
    wo = nc.dram_tensor("wo", [H, 128, DIM], BF16, kind="ExternalInput")
    wob = nc.dram_tensor("wob", [1, DIM], F32, kind="ExternalInput")
    maskblk = nc.dram_tensor("maskblk", [max(nmask, 1), 128, 512], BF16,
                             kind="ExternalInput")
    out_c = nc.dram_tensor("out", [R, DIM], F32, kind="ExternalOutput")

    LAT = KVLR + ROPE  # 576

    with tile.TileContext(nc) as tc:
        with tc.tile_pool(name="konst", bufs=1) as konst, \
             tc.tile_pool(name="dram", bufs=1, space="DRAM") as dram:

            # ---- constants ----
            ones_f = konst.tile([128, 1], F32)
            nc.vector.memset(ones_f[:, :], 1.0)
            ones_col = konst.tile([128, 1], F32R)
            nc.vector.tensor_copy(out=ones_col[:, :], in_=ones_f[:, :])
            ones_colb = konst.tile([128, 1], BF16)
            nc.vector.tensor_copy(out=ones_colb[:, :], in_=ones_f[:, :])
            ones_rf = konst.tile([1, 128], F32)
            nc.vector.memset(ones_rf[:, :], 1.0)
            ones_row = konst.tile([1, 128], F32R)
            nc.vector.tensor_copy(out=ones_row[:, :], in_=ones_rf[:, :])
            ones_rowb = konst.tile([1, 128], BF16)
            nc.vector.tensor_copy(out=ones_rowb[:, :], in_=ones_rf[:, :])
            eps_t = konst.tile([1, 1], F32)
            nc.vector.memset(eps_t[:, :], EPS)
            ab_t = konst.tile([128, 256], BF16)
            nc.sync.dma_start(out=ab_t, in_=abperm[:, :])

            for _rep in range(repeat):
                qkv = tc.alloc_tile_pool(name=f"qkv{_rep}", bufs=1)
                # ---- collective DRAM tiles (bf16 payloads) ----
                ag_in = dram.tile([LATM * 128, R], BF16, name=f"ag_in{_rep}")
                ag_out = dram.tile([NC, LATM * 128, R], BF16,
                                   addr_space=("Local" if sim_mode else "Shared"),
                                   name=f"ag_out{_rep}")
                a2a_in = dram.tile([NC, HPC * VD, R], BF16, name=f"a2a_in{_rep}")
                a2a_out = dram.tile([NC, HPC * VD, R], BF16, name=f"a2a_out{_rep}")
                # ---- persistent q/k/v (phase 2 -> phase 3) ----
                # q/k in fp8e4m3 with DoubleRow layout: plane 0 = nope (128
                # contraction rows), plane 1 = rope (rows 0-63; 64-127 zero).
                q8 = [qkv.tile([128, 2, N], FP8, name=f"q8_{i}") for i in range(HPC)]
                k8 = [qkv.tile([128, 2, N], FP8, name=f"k8_{i}") for i in range(HPC)]
                for t8 in (*q8, *k8):
                    nc.vector.memset(t8[64:128, 1, :], 0.0)
                vt = qkv.tile([128, N // 128, HPC * VD], BF16)   # token-major V

                # ================= PHASE 1: latent down-proj (row shard) ============
                with tc.tile_pool(name=f"p1sb{_rep}", bufs=1) as p1sb, \
                     tc.tile_pool(name=f"p1act{_rep}", bufs=1) as p1act, \
                     tc.tile_pool(name=f"p1tmp{_rep}", bufs=2) as p1tmp, \
                     tc.tile_pool(name=f"p1ps{_rep}", bufs=1, space="PSUM") as p1ps, \
                     tc.tile_pool(name=f"p1ps2{_rep}", bufs=1, space="PSUM") as p1ps2, \
                     tc.tile_pool(name=f"p1pr{_rep}", bufs=1, space="PSUM") as p1pr, \
                     tc.tile_pool(name=f"p1ps1{_rep}", bufs=1, space="PSUM") as p1ps1:

                    t1 = p1sb.tile([64, 2 * R], F32)   # [cos;sin],[sin;cos] blocks
                    nc.sync.dma_start(out=t1, in_=trig1[:, :])
                    # x and w1 load in 4 k-chunks on separate queues so the PE
                    # can start after the first chunk lands
                    KC = 4
                    xall = p1sb.tile([128, KD, R], BF16, name=f"xall{_rep}")
                    w1_sb = p1sb.tile([128, KD, QLR + LAT], BF16, name=f"w1sb{_rep}")
                    for kc in range(KC):
                        ks = slice(kc * (KD // KC), (kc + 1) * (KD // KC))
                        nc.scalar.dma_start(
                            out=xall[:, ks, :],
                            in_=x_c[ks].rearrange("k p t -> p k t"))
                        nc.gpsimd.dma_start(
                            out=w1_sb[:, ks, :],
                            in_=w1[ks].rearrange("k p c -> p k c"))
                    nw_sb = p1sb.tile([128, 8], F32, name=f"nw{_rep}")
                    nc.sync.dma_start(out=nw_sb, in_=normw[:, :])
                    if flags['ba']:
                        b1_sb = p1sb.tile([128, LATM], F32, name=f"b1sb{_rep}")
                        nc.sync.dma_start(out=b1_sb, in_=b1[:, :])
                    # staging for the AllGather payload
                    nrm_all = p1sb.tile([128, LATM, R], BF16, name=f"nrm{_rep}")
                    # rows 64-127 of the rope chunk are never written but do
                    # get transported by the AllGather
                    nc.vector.memset(nrm_all[64:128, 8, :], 0.0)

                    for path in range(2):  # 0: q, 1: kv
                        coff = 0 if path == 0 else QLR
                        # k-outer accumulation: 4-5 concurrent PSUM groups, so
                        # compute can start as soon as the first x/w1 chunk lands
                        pss = []
                        for m in range(5 if path == 1 else 4):
                            if m == 4:
                                ps = p1pr.tile([64, R], F32, tag="p1rope",
                                               name=f"psr{path}{m}")
                            else:
                                ps = p1ps.tile([128, R], F32, tag=f"p1acc{m}",
                                               name=f"ps{path}{m}")
                            pss.append(ps)
                        for k in range(KD):
                            for m, ps in enumerate(pss):
                                mp = 64 if m == 4 else 128
                                nc.tensor.matmul(
                                    ps[:, :],
                                    w1_sb[:, k, coff + m * 128:coff + m * 128 + mp],
                                    xall[:, k, :],
                                    start=(k == 0), stop=(k == KD - 1))

                        acts = []
                        sums_ps = p1ps1.tile([1, R], F32, name=f"sums{path}",
                                             tag="sums")
                        for m in range(5 if path == 1 else 4):
                            mp = 64 if m == 4 else 128
                            ps = pss[m]
                            a = p1act.tile([128, R], BF16, tag=f"act{m}",
                                           name=f"a{path}{m}")
                            if flags['ba']:
                                nc.vector.tensor_scalar_add(
                                    a[0:mp, :], ps[0:mp, :],
                                    b1_sb[0:mp, path * 4 + m:path * 4 + m + 1])
                            else:
                                nc.scalar.activation(out=a[0:mp, :], in_=ps[0:mp, :],
                                                     func=AF.Copy)
                            acts.append(a)
                            if m < 4:   # latent chunks: accumulate sum of squares
                                sq = p1tmp.tile([128, R], F32R, tag="sq")
                                nc.vector.tensor_mul(sq[:, :], a[:, :], a[:, :])
                                nc.tensor.matmul(sums_ps[:, :], ones_col[:, :], sq[:, :],
                                                 start=(m == 0), stop=(m == 3),
                                                 skip_group_check=True)
                        # rstd = 1/sqrt(mean + eps), broadcast across partitions
                        std = p1tmp.tile([1, R], F32, tag="std")
                        nc.scalar.activation(out=std[:, :], in_=sums_ps[:, :],
                                             func=AF.Sqrt,
                                             scale=1.0 / (QLR if path == 0 else KVLR),
                                             bias=eps_t[:, :])
                        rstd_f = p1tmp.tile([1, R], F32, tag="rstdf")
                        nc.vector.reciprocal(out=rstd_f[:, :], in_=std[:, :])
                        rstd_bc = p1tmp.tile([128, R], F32, tag="rstdbc")
                        nc.gpsimd.partition_broadcast(rstd_bc[:, :], rstd_f[:, :],
                                                      channels=128)
                        for m in range(4):
                            nc.vector.scalar_tensor_tensor(
                                out=nrm_all[:, path * 4 + m, :], in0=acts[m][:, :],
                                scalar=nw_sb[:, path * 4 + m:path * 4 + m + 1],
                                in1=rstd_bc[:, :],
                                op0=ALU.mult, op1=ALU.mult)
                        if path == 1:   # rope on k_pe chunk [64, R]
                            kpe = acts[4]
                            u = p1tmp.tile([64, R], BF16, tag="u1")
                            nc.vector.tensor_mul(u[:, :], kpe[0:64, :], t1[:, 0:R])
                            w = p1tmp.tile([64, R], BF16, tag="w1t")
                            nc.vector.tensor_mul(w[:, :], kpe[0:64, :], t1[:, R:2 * R])
                            rot = p1ps2.tile([64, R], F32, tag="p1rep", name="rotps")
                            nc.tensor.matmul(rot[:, :], ab_t[0:64, 0:64], u[:, :],
                                             start=True, stop=False)
                            nc.tensor.matmul(rot[:, :], ab_t[0:64, 128:192], w[:, :],
                                             start=False, stop=True)
                            nc.scalar.activation(out=nrm_all[0:64, 8, :],
                                                 in_=rot[:, :], func=AF.Copy)
                    nc.sync.dma_start(
                        out=ag_in.rearrange("(m p) t -> p m t", p=128),
                        in_=nrm_all[:, :, :])

                # ---- AllGather ----
                if sim_mode:
                    nc.sync.dma_start(out=ag_out[0, 0:1, :], in_=ag_in[0:1, :])
                else:
                    nc.gpsimd.collective_compute(
                        "AllGather", ALU.bypass,
                        replica_groups=[list(range(NC))],
                        ins=[ag_in.opt()], outs=[ag_out.opt()])

                # prefetch mask blocks (phase 3) on the Pool queue. Own pool:
                # released before the AllToAll so the next rep's phase 1 can
                # overlap this rep's AllToAll + phase 4.
                late = tc.alloc_tile_pool(name=f"late{_rep}", bufs=1)
                mtile = late.tile([128, max(nmask, 1), 512], BF16,
                                  name=f"mt{_rep}")
                nc.gpsimd.dma_start(
                    out=mtile,
                    in_=maskblk.rearrange("n p t -> p n t"))

                # ================= PHASE 2: per-head up-projections ================
                with tc.tile_pool(name=f"p2w{_rep}", bufs=1) as p2w, \
                     tc.tile_pool(name=f"p2lat{_rep}", bufs=2) as p2lat, \
                     tc.tile_pool(name=f"p2tmp{_rep}", bufs=2) as p2tmp, \
                     tc.tile_pool(name=f"p2ps{_rep}", bufs=3, space="PSUM") as p2ps, \
                     tc.tile_pool(name=f"p2psv{_rep}", bufs=2, space="PSUM") as p2psv, \
                     tc.tile_pool(name=f"p2pr{_rep}", bufs=2, space="PSUM") as p2pr:

                    tq = p2w.tile([128, 2 * S], BF16)
                    nc.scalar.dma_start(out=tq, in_=trigq[:, :])
                    w2_sb = p2w.tile([128, 4, 5 * 128 + HPC * VD], BF16,
                                     name=f"w2sb{_rep}")
                    nc.scalar.dma_start(out=w2_sb, in_=w2.rearrange("k p c -> p k c"))
                    if flags['bvb']:
                        bvb_t = p2w.tile([1, HPC * VD], F32R)
                        nc.sync.dma_start(out=bvb_t, in_=bvb[:, :].bitcast(F32R))
                    if flags['b2']:
                        b2_sb = p2w.tile([128, 5], F32)
                        nc.sync.dma_start(out=b2_sb, in_=b2[:, :])

                    for s in range(NBLK):
                        tsl = slice(s * R, (s + 1) * R)
                        pos = (s % (S // R)) * R       # position within batch
                        lat = p2lat.tile([128, LATM, R], BF16, tag="lat",
                                         name=f"lat{s}")
                        nc.sync.dma_start(
                            out=lat,
                            in_=ag_out[s].rearrange("(m p) t -> p m t", p=128))
                        for m in range(3):
                            ps = p2ps.tile([128, R], F32, tag="p2acc")
                            for k in range(4):
                                nc.tensor.matmul(ps[:, :],
                                                 w2_sb[:, k, m * 128:(m + 1) * 128],
                                                 lat[:, k, :],
                                                 start=(k == 0), stop=(k == 3))
                            if m < 2:
                                if flags['b2']:
                                    nc.vector.tensor_scalar_add(q8[m][:, 0, tsl],
                                                                ps[:, :],
                                                                b2_sb[:, m:m + 1])
                                else:
                                    nc.scalar.activation(out=q8[m][:, 0, tsl],
                                                         in_=ps[:, :], func=AF.Copy)
                            else:
                                rst = p2tmp.tile([128, R], F32, tag="rst")
                                if flags['b2']:
                                    nc.vector.tensor_scalar_add(rst[:, :], ps[:, :],
                                                                b2_sb[:, 2:3])
                                else:
                                    nc.scalar.activation(out=rst[:, :], in_=ps[:, :],
                                                         func=AF.Copy)
                                u = p2tmp.tile([128, R], BF16, tag="u2")
                                nc.vector.tensor_mul(u[:, :], rst[:, :],
                                                     tq[:, pos:pos + R])
                                w = p2tmp.tile([128, R], BF16, tag="w2")
                                nc.vector.tensor_mul(w[:, :], rst[:, :],
                                                     tq[:, S + pos:S + pos + R])
                                for lh in range(HPC):
                                    rps = p2pr.tile([64, R], F32, tag="p2rope",
                                                    name=f"rps{lh}")
                                    nc.tensor.matmul(
                                        rps[:, :], ab_t[:, lh * 64:(lh + 1) * 64],
                                        u[:, :], start=True, stop=False)
                                    nc.tensor.matmul(
                                        rps[:, :],
                                        ab_t[:, 128 + lh * 64:128 + (lh + 1) * 64],
                                        w[:, :], start=False, stop=True)
                                    nc.scalar.activation(out=q8[lh][0:64, 1, tsl],
                                                         in_=rps[:, :], func=AF.Copy)
                        # k_nope
                        for m in range(2):
                            ps = p2ps.tile([128, R], F32, tag="p2acc")
                            for k in range(4):
                                nc.tensor.matmul(
                                    ps[:, :],
                                    w2_sb[:, k, (3 + m) * 128:(4 + m) * 128],
                                    lat[:, 4 + k, :],
                                    start=(k == 0), stop=(k == 3))
                            if flags['b2']:
                                nc.vector.tensor_scalar_add(k8[m][:, 0, tsl], ps[:, :],
                                                            b2_sb[:, 3 + m:4 + m])
                            else:
                                nc.scalar.activation(out=k8[m][:, 0, tsl], in_=ps[:, :],
                                                     func=AF.Copy)
                        # v (token-major)
                        for mt in range(4):
                            ps = p2psv.tile([128, HPC * VD], F32, tag="p2v")
                            if flags['bvb']:
                                nc.tensor.matmul(ps[:, :], ones_row[:, :], bvb_t[:, :],
                                                 start=True, stop=False)
                            for k in range(4):
                                nc.tensor.matmul(
                                    ps[:, :],
                                    lat[:, 4 + k, mt * 128:(mt + 1) * 128],
                                    w2_sb[:, k, 5 * 128:5 * 128 + HPC * VD],
                                    start=(k == 0 and not flags['bvb']),
                                    stop=(k == 3))
                            nc.vector.tensor_copy(out=vt[:, s * 4 + mt, :],
                                                  in_=ps[:, :])
                        # k_pe: same 64 rows go into both heads' rope plane
                        for lh in range(HPC):
                            nc.vector.tensor_copy(out=k8[lh][0:64, 1, tsl],
                                                  in_=lat[0:64, 8, :])

                # ================= PHASE 3: attention =============================
                with tc.tile_pool(name=f"p3p{_rep}", bufs=3) as p3p, \
                     tc.tile_pool(name=f"p3o{_rep}", bufs=2) as p3o, \
                     tc.tile_pool(name=f"p3ao{_rep}", bufs=2) as p3ao, \
                     tc.tile_pool(name=f"p3sc{_rep}", bufs=2, space="PSUM") as p3sc, \
                     tc.tile_pool(name=f"p3out{_rep}", bufs=2, space="PSUM") as p3out, \
                     tc.tile_pool(name=f"p3rs{_rep}", bufs=2, space="PSUM") as p3rs:

                    for b in range(B):
                        for qc in range(S // 512):
                            qsl = slice(b * S + qc * 512, b * S + (qc + 1) * 512)
                            ao2 = p3ao.tile([128, HPC, 512], BF16, tag="ao2")
                            for lh in range(HPC):
                                out_ps = p3out.tile([128, 512], F32, tag="outp")
                                rs_ps = p3rs.tile([1, 512], F32, tag="rsp")
                                kbs = [kb for kb in range(S // 128)
                                       if cls[qc][kb] != SKIP]
                                i = 0
                                while i < len(kbs):
                                    npair = min(2, len(kbs) - i)
                                    sc2 = p3sc.tile([128, 2, 512], F32, tag="sc2")
                                    for j in range(npair):
                                        kb = kbs[i + j]
                                        ksl = slice(b * S + kb * 128,
                                                    b * S + kb * 128 + 128)
                                        # fp8 DoubleRow: 192-dim contraction in
                                        # one matmul at 0.5 cycles/row
                                        nc.tensor.matmul(
                                            sc2[:, j, :],
                                            k8[lh][:, :, ksl], q8[lh][:, :, qsl],
                                            start=True, stop=True,
                                            perf_mode=mybir.MatmulPerfMode.DoubleRow,
                                            skip_group_check=True)
                                    P2 = p3p.tile([128, 2, 512], BF16, tag="P2")
                                    nc.scalar.activation(
                                        out=P2[:, 0:npair, :],
                                        in_=sc2[:, 0:npair, :],
                                        func=AF.Exp, scale=SCALE)
                                    for j in range(npair):
                                        kb = kbs[i + j]
                                        if cls[qc][kb] >= 0:
                                            nc.vector.tensor_mul(
                                                P2[:, j, :], P2[:, j, :],
                                                mtile[:, cls[qc][kb], :])
                                    for j in range(npair):
                                        kb = kbs[i + j]
                                        last = (i + j == len(kbs) - 1)
                                        nc.tensor.matmul(
                                            out_ps[:, :],
                                            vt[:, b * 16 + kb, lh * VD:(lh + 1) * VD],
                                            P2[:, j, :], start=(i + j == 0), stop=last,
                                            skip_group_check=True)
                                        nc.tensor.matmul(
                                            rs_ps[:, :], ones_colb[:, :],
                                            P2[:, j, :], start=(i + j == 0), stop=last,
                                            skip_group_check=True)
                                    i += npair
                                inv_f = p3o.tile([1, 512], F32, tag="invf")
                                nc.vector.reciprocal(out=inv_f[:, :], in_=rs_ps[:, :])
                                bc = p3o.tile([128, 512], F32, tag="bc")
                                nc.gpsimd.partition_broadcast(bc[:, :], inv_f[:, :],
                                                              channels=128)
                                nc.vector.tensor_mul(ao2[:, lh, :], out_ps[:, :],
                                                     bc[:, :])
                            nc.sync.dma_start(
                                out=a2a_in[b * 4 + qc].rearrange(
                                    "(h p) t -> p h t", p=128),
                                in_=ao2[:, :, :])

                late.release()
                qkv.release()

                # ---- AllToAll ----
                if sim_mode:
                    nc.sync.dma_start(out=a2a_out[0, 0:1, :], in_=a2a_in[0, 0:1, :])
                else:
                    nc.gpsimd.collective_compute(
                        "AllToAll", ALU.bypass,
                        replica_groups=[list(range(NC))],
                        ins=[a2a_in.opt()], outs=[a2a_out.opt()])

                # ================= PHASE 4: output projection =====================
                with tc.tile_pool(name=f"p4l{_rep}", bufs=1) as p4l, \
                     tc.tile_pool(name=f"p4w{_rep}", bufs=2) as p4w, \
                     tc.tile_pool(name=f"p4o{_rep}", bufs=3) as p4o, \
                     tc.tile_pool(name=f"p4ps{_rep}", bufs=4, space="PSUM") as p4ps:

                    lt = p4l.tile([128, H, 512], BF16, name=f"lt{_rep}")
                    nc.sync.dma_start(
                        out=lt,
                        in_=a2a_out.rearrange("c (h p) t -> p (c h) t", p=128))
                    if flags['wob']:
                        wob_t = p4l.tile([1, DIM], F32R)
                        nc.sync.dma_start(out=wob_t, in_=wob[:, :].bitcast(F32R))
                    for n_ in range(4):
                        wchunk = p4w.tile([128, H, 512], BF16, tag="wo",
                                          name=f"wo{n_}")
                        nc.scalar.dma_start(
                            out=wchunk,
                            in_=wo[:, :, n_ * 512:(n_ + 1) * 512]
                            .rearrange("k p c -> p k c"))
                        for m in range(4):
                            ps = p4ps.tile([128, 512], F32, tag="p4acc")
                            if flags['wob']:
                                nc.tensor.matmul(ps[:, :], ones_row[:, :],
                                                 wob_t[:, n_ * 512:(n_ + 1) * 512],
                                                 start=True, stop=False)
                            for k in range(H):
                                nc.tensor.matmul(
                                    ps[:, :],
                                    lt[:, k, m * 128:(m + 1) * 128],
                                    wchunk[:, k, :],
                                    start=(k == 0 and not flags['wob']),
                                    stop=(k == H - 1))
                            ob = p4o.tile([128, 512], F32, tag="ob")
                            nc.scalar.activation(out=ob[:, :], in_=ps[:, :],
                                                 func=AF.Copy)
                            nc.sync.dma_start(
                                out=out_c[m * 128:(m + 1) * 128,
                                          n_ * 512:(n_ + 1) * 512],
                                in_=ob[:, :])

    nc.finalize()
    return nc


_ROPE_PERM = np.concatenate([np.arange(0, ROPE, 2), np.arange(1, ROPE, 2)])

_CACHE = {}


def _prep_inputs(inputs):
    """Host-side slicing/permutation -> (schedule key data, per-core in_maps)."""
    x = np.ascontiguousarray(np.asarray(inputs['x'], np.float32).reshape(N, DIM))
    mask = np.asarray(inputs['mask'])
    cls, blocks = _classify_mask(mask)

    cos_t, sin_t = _rope_tables()            # [32, S]
    trigq = np.concatenate(
        [np.concatenate([cos_t, sin_t, cos_t, sin_t], 0),
         np.concatenate([sin_t, cos_t, sin_t, cos_t], 0)], 1)   # [128, 2S]

    A128, B128 = _perm_mats(128)
    abperm = np.concatenate([A128, B128], 1).astype(NPBF16)     # [128, 256]

    wq_a = np.asarray(inputs['wq_a_w'], np.float32)            # [QLR, DIM]
    wkv_a = np.asarray(inputs['wkv_a_w'], np.float32)          # [KVLR+ROPE, DIM]
    wkv_a_p = np.concatenate([wkv_a[:KVLR], wkv_a[KVLR:][_ROPE_PERM]], 0)
    bqa = np.asarray(inputs['wq_a_b'], np.float32)
    bkva = np.asarray(inputs['wkv_a_b'], np.float32)
    bkva_p = np.concatenate([bkva[:KVLR], bkva[KVLR:][_ROPE_PERM]], 0)
    # b1 as [128, 9]: chunks 0-3 q bias, 4-7 kv latent bias, 8 rope bias (64 rows)
    b1 = np.zeros((128, LATM), np.float32)
    b1[:, 0:4] = bqa.reshape(4, 128).T
    b1[:, 4:8] = bkva_p[:KVLR].reshape(4, 128).T
    b1[0:64, 8] = bkva_p[KVLR:]

    normw = np.concatenate([np.asarray(inputs['q_norm_w'], np.float32),
                            np.asarray(inputs['kv_norm_w'], np.float32)])[None, :]

    # w1: [16, 128, 1088] bf16 = (wq_a | wkv_a | rope rows) transposed
    w1 = np.concatenate([wq_a.T, wkv_a_p.T], 1).reshape(KD, 128, QLR + KVLR + ROPE)
    w1 = np.ascontiguousarray(w1).astype(NPBF16)

    wq_b = np.asarray(inputs['wq_b_w'], np.float32).reshape(H, QKD, QLR)
    bq_b = np.asarray(inputs['wq_b_b'], np.float32).reshape(H, QKD)
    wkv_b = np.asarray(inputs['wkv_b_w'], np.float32).reshape(H, NOPE + VD, KVLR)
    bkv_b = np.asarray(inputs['wkv_b_b'], np.float32).reshape(H, NOPE + VD)
    wo = np.asarray(inputs['wo_w'], np.float32)                # [DIM, H*VD]
    # wo_sb layout: [head-slice k, vd part, 2048]
    wo_p = np.ascontiguousarray(
        wo.T.reshape(H, VD, DIM)).astype(NPBF16)

    shared = {
        'w1': w1,
        'b1': b1,
        'normw': normw,
        'trigq': np.ascontiguousarray(trigq).astype(NPBF16),
        'abperm': abperm,
        'wo': wo_p,
        'wob': np.asarray(inputs['wo_b'], np.float32)[None, :],
        'maskblk': blocks if len(blocks) else np.zeros((1, 128, 512), NPBF16),
    }

    in_maps = []
    for c in range(NC):
        h0, h1 = 2 * c, 2 * c + 1
        pos_c = (c % (S // R)) * R
        # q_b rows: h0 nope, h1 nope, [h0 rope-e, h0 rope-o, h1 rope-e, h1 rope-o]
        wqb_c = np.concatenate([
            wq_b[h0, :NOPE], wq_b[h1, :NOPE],
            wq_b[h0, NOPE:][_ROPE_PERM], wq_b[h1, NOPE:][_ROPE_PERM]], 0)
        bqb_c = np.concatenate([
            bq_b[h0, :NOPE], bq_b[h1, :NOPE],
            bq_b[h0, NOPE:][_ROPE_PERM], bq_b[h1, NOPE:][_ROPE_PERM]], 0)
        wkb_c = np.concatenate([wkv_b[h0, :NOPE], wkv_b[h1, :NOPE]], 0)
        bkb_c = np.concatenate([bkv_b[h0, :NOPE], bkv_b[h1, :NOPE]], 0)
        wvb_c = np.concatenate([wkv_b[h0, NOPE:], wkv_b[h1, NOPE:]], 0)
        bvb_c = np.concatenate([bkv_b[h0, NOPE:], bkv_b[h1, NOPE:]], 0)
        # w2: [4, 128, 3*128 | 2*128 | 256]
        w2 = np.concatenate([
            wqb_c.T.reshape(4, 128, 3 * 128),
            wkb_c.T.reshape(4, 128, 2 * 128),
            wvb_c.T.reshape(4, 128, HPC * VD)], 2)
        b2 = np.concatenate([bqb_c.reshape(3, 128).T,
                             bkb_c.reshape(2, 128).T], 1)      # [128, 5]
        m = dict(shared)
        m.update({
            'x_c': np.ascontiguousarray(x[c * R:(c + 1) * R].T).reshape(KD, 128, R).astype(NPBF16),
            'w2': np.ascontiguousarray(w2).astype(NPBF16),
            'b2': b2,
            'bvb': bvb_c[None, :],
            'trig1': np.ascontiguousarray(np.concatenate([
                np.concatenate(
                    [cos_t[:, pos_c:pos_c + R], sin_t[:, pos_c:pos_c + R]], 0),
                np.concatenate(
                    [sin_t[:, pos_c:pos_c + R], cos_t[:, pos_c:pos_c + R]], 0)], 1)),
        })
        in_maps.append(m)
    return cls, in_maps


class _Runner:
    """Compile once, execute many times on the 8 axon-tunneled NeuronCores."""

    def __init__(self, nc):
        import jax
        from jax.experimental.shard_map import shard_map
        from jax.sharding import Mesh, PartitionSpec
        from concourse import bass2jax, mybir as _mybir
        bass2jax.install_neuronx_cc_hook()
        self.jax = jax
        in_names, out_names, out_avals, zero_outs = [], [], [], []
        partition_name = (nc.partition_id_tensor.name
                          if nc.partition_id_tensor else None)
        for alloc in nc.m.functions[0].allocations:
            if not isinstance(alloc, _mybir.MemoryLocationSet):
                continue
            name = alloc.memorylocations[0].name
            if alloc.kind == "ExternalInput":
                if name != partition_name:
                    in_names.append(name)
            elif alloc.kind == "ExternalOutput":
                shape = tuple(alloc.tensor_shape)
                dtype = _mybir.dt.np(alloc.dtype)
                out_names.append(name)
                out_avals.append(jax.core.ShapedArray(shape, dtype))
                zero_outs.append(np.zeros(shape, dtype))
        self.n_params = len(in_names)
        self.in_names = list(in_names)
        self.out_names = out_names
        self.out_avals = out_avals
        self.zero_outs = zero_outs
        all_in = in_names + out_names
        if partition_name is not None:
            all_in.append(partition_name)

        def _body(*args):
            operands = list(args)
            if partition_name is not None:
                operands.append(bass2jax.partition_id_tensor())
            outs = bass2jax._bass_exec_p.bind(
                *operands,
                out_avals=tuple(out_avals),
                in_names=tuple(all_in),
                out_names=tuple(out_names),
                lowering_input_output_aliases=(),
                sim_require_finite=True,
                sim_require_nnan=True,
                nc=nc)
            return tuple(outs)

        devices = jax.devices()[:NC]
        self.mesh = Mesh(np.asarray(devices), ("core",))
        n_out = len(out_names)
        in_specs = (PartitionSpec("core"),) * (self.n_params + n_out)
        out_specs = (PartitionSpec("core"),) * n_out
        donate = tuple(range(self.n_params, self.n_params + n_out))
        self.fn = jax.jit(
            shard_map(_body, mesh=self.mesh, in_specs=in_specs,
                      out_specs=out_specs, check_rep=False),
            donate_argnums=donate, keep_unused=True)

    def concat_inputs(self, in_maps):
        return [np.concatenate([np.asarray(in_maps[c][nm])
                                for c in range(NC)], axis=0)
                for nm in self.in_names]

    def zeros(self):
        return [np.zeros((NC * z.shape[0], *z.shape[1:]), z.dtype)
                for z in self.zero_outs]

    def __call__(self, concat_in, concat_zeros):
        out = self.fn(*concat_in, *concat_zeros)
        return out

    def run(self, in_maps):
        outs = self(self.concat_inputs(in_maps), self.zeros())
        res = []
        for c in range(NC):
            res.append({nm: np.asarray(outs[i]).reshape(NC, *self.out_avals[i].shape)[c]
                        for i, nm in enumerate(self.out_names)})
        return res


def _flags(inputs):
    return {
        'ba': bool(np.any(inputs['wq_a_b'])) or bool(np.any(inputs['wkv_a_b'])),
        'b2': (bool(np.any(inputs['wq_b_b'])) or
               bool(np.any(np.asarray(inputs['wkv_b_b'])
                           .reshape(H, NOPE + VD)[:, :NOPE]))),
        'bvb': bool(np.any(np.asarray(inputs['wkv_b_b'])
                           .reshape(H, NOPE + VD)[:, NOPE:])),
        'wob': bool(np.any(inputs['wo_b'])),
    }


def _get_exec(cls, nmask, flags):
    key = (tuple(tuple(r) for r in cls), nmask,
           flags['ba'], flags['b2'], flags['bvb'], flags['wob'])
    if key not in _CACHE:
        nc = _build(cls, nmask, flags)
        _CACHE[key] = _Runner(nc)
    return _CACHE[key]


def kernel(**inputs):
    cls, in_maps = _prep_inputs(inputs)
    nmask = max(len(in_maps[0]['maskblk']), 1)
    flags = _flags(inputs)
    runner = _get_exec(cls, nmask, flags)
    results = runner.run(in_maps)
    out = np.concatenate([results[c]["out"] for c in range(NC)], 0)
    return out.reshape(B, S, DIM)
